# revision 1
# baseline (speedup 1.0000x reference)
"""Scaled-cosine attention (SwinV2-style) Trainium2 kernel.

Full inputs -> full output. Internally: data-parallel over batch N=8,
one batch element per NeuronCore, no collectives.

Per-core math (L=1024 tokens, C=768, H=12 heads, D=64):
  qkv = x @ W_in^T (+b);  q,k per head L2-normalized; attn = softmax(
  cos(q,k) * exp(min(logit_scale, log(100))));  o = (attn @ v) * head_scale;
  y = o @ W_out^T (+out_b)

Layout strategy (fp32 data; matmuls in fp32r = full-rate ~tf32 rounding):
  - Q^T,K^T computed directly as qkv^T j-tiles (lhsT = W^T tiles, rhs = x^T)
  - V computed in row layout (lhsT = x^T tiles, rhs = W_v^T), stored with a
    ones column per head so attn@V also produces the softmax denominator
  - q/k inverse norms via block-ones matmuls, processed per head-pair so
    attention can start before all of qkv finishes
  - 1/||q|| broadcast across partitions (gpsimd) and folded into Qhat
    together with the logit scale (one scalar_tensor_tensor per head)
  - 1/||k|| moved to per-key-partition layout via a DRAM bounce and folded
    into the exp() activation's per-partition scale
  - attn@V accumulates over key tiles in PSUM; output rows scaled by
    head_scale/denominator during PSUM eviction (scalar_tensor_tensor)
"""

import math
import sys

import numpy as np

_REPO = "/opt/trn_rl_repo"
if _REPO not in sys.path:
    sys.path.insert(0, _REPO)

import concourse.bacc as bacc
import concourse.mybir as mybir
import concourse.tile as tile
from concourse import bass_utils

L = 1024
C = 768
H = 12
D = 64
NKT = C // 128          # 6 contraction tiles
NLT = L // 128          # 8
LOG_MAX = math.log(1.0 / 0.01)
F32 = mybir.dt.float32
F32R = mybir.dt.float32r
EXP = mybir.ActivationFunctionType.Exp
MULT = mybir.AluOpType.mult


def _r(ap):
    return ap if ap.dtype == F32R else ap.bitcast(F32R)


def build(nc, has_b1, debug=False):
    xt = nc.dram_tensor("xt", (C, L), F32, kind="ExternalInput").ap()
    wt = nc.dram_tensor("wt", (C, 3 * C), F32, kind="ExternalInput").ap()
    owt = nc.dram_tensor("owt", (C, C), F32, kind="ExternalInput").ap()
    lsc = nc.dram_tensor("lsc", (1, H), F32, kind="ExternalInput").ap()
    hsc = nc.dram_tensor("hsc", (1, H), F32, kind="ExternalInput").ap()
    bonesd = nc.dram_tensor("bones", (128, 33), F32, kind="ExternalInput").ap()
    onescd = nc.dram_tensor("onesc", (128, H), F32, kind="ExternalInput").ap()
    if has_b1:
        b1 = nc.dram_tensor("b1", (1, 3 * C), F32, kind="ExternalInput").ap()
        ones512d = nc.dram_tensor("ones512", (1, 512), F32,
                                  kind="ExternalInput").ap()
    y = nc.dram_tensor("y", (L, C), F32, kind="ExternalOutput").ap()
    if debug:
        dbg = {nm: nc.dram_tensor(nm, shp, F32, kind="ExternalOutput").ap()
               for nm, shp in [("yqt", (128, 6 * L)), ("ykt", (128, 6 * L)),
                               ("yvt", (128, NLT * H * 65)),
                               ("ynorms", (128, 6 * L)), ("yrk", (128, H * 8)),
                               ("yqhat", (128, L)), ("yet", (128, L)),
                               ("yot", (128, 6 * L)), ("ydn", (1, L))]}

    with tile.TileContext(nc) as tc:
        with (
            tc.tile_pool(name="wq", bufs=9 if has_b1 else 12) as p_wq,
            tc.tile_pool(name="w", bufs=6) as p_w,
            tc.tile_pool(name="xo", bufs=1) as p_xo,
            tc.tile_pool(name="qk", bufs=1) as p_qk,
            tc.tile_pool(name="ot", bufs=1) as p_ot,
            tc.tile_pool(name="v", bufs=1) as p_v,
            tc.tile_pool(name="n", bufs=1) as p_n,
            tc.tile_pool(name="e", bufs=4 if has_b1 else 6) as p_e,
            tc.tile_pool(name="b", bufs=6) as p_b,
            tc.tile_pool(name="d", bufs=2) as p_d,
            tc.tile_pool(name="c", bufs=1) as p_c,
            tc.tile_pool(name="dram", bufs=1, space="DRAM") as p_dram,
            tc.tile_pool(name="q", bufs=2, space="PSUM") as ps_q,
            tc.tile_pool(name="s", bufs=2, space="PSUM") as ps_s,
            tc.tile_pool(name="o", bufs=2, space="PSUM") as ps_o,
        ):
            # ---------------- input DMAs -----------------
            xt6 = p_xo.tile([128, NKT * L], F32R, tag="xo")
            wqk = {}

            def load_wqk_pair(p):
                # per-pair weight columns: [:, 0:128] = Q col p of tile kt,
                # [:, 128:256] = K col p
                for kt in range(NKT):
                    t = p_wq.tile([128, 256], F32R, tag="wq",
                                  name=f"wqk{p}_{kt}")
                    nc.sync.dma_start(
                        t[:, 0:128],
                        wt[kt * 128:(kt + 1) * 128,
                           p * 128:p * 128 + 128].bitcast(F32R))
                    nc.sync.dma_start(
                        t[:, 128:256],
                        wt[kt * 128:(kt + 1) * 128,
                           C + p * 128:C + p * 128 + 128].bitcast(F32R))
                    wqk[(p, kt)] = t

            # interleave x^T blocks, pair-0 weight columns AND V weights so
            # the first qkv chain starts as soon as block 0 lands and the
            # V-part chains right behind pair 0 without waiting on wv
            wv = []
            for kt in range(NKT):
                nc.sync.dma_start(xt6[:, kt * L:(kt + 1) * L],
                                  xt[kt * 128:(kt + 1) * 128, :].bitcast(F32R))
                t = p_wq.tile([128, 256], F32R, tag="wq", name=f"wqk0_{kt}")
                nc.sync.dma_start(
                    t[:, 0:128],
                    wt[kt * 128:(kt + 1) * 128, 0:128].bitcast(F32R))
                nc.sync.dma_start(
                    t[:, 128:256],
                    wt[kt * 128:(kt + 1) * 128, C:C + 128].bitcast(F32R))
                wqk[(0, kt)] = t
                tv = p_w.tile([128, C], F32R, tag="w", name=f"wv{kt}")
                nc.sync.dma_start(
                    tv[:], wt[kt * 128:(kt + 1) * 128, 2 * C:3 * C].bitcast(F32R))
                wv.append(tv)
            lsrow = p_c.tile([1, H], F32, tag="lsr")
            nc.sync.dma_start(lsrow[:], lsc[:])
            hsrow = p_c.tile([1, H], F32, tag="hsr")
            nc.sync.dma_start(hsrow[:], hsc[:])
            bones = p_c.tile([128, 33], F32R, tag="bones")
            nc.sync.dma_start(bones[:], bonesd[:].bitcast(F32R))
            if has_b1:
                brow = p_c.tile([1, 3 * C], F32R, tag="b1r")
                nc.sync.dma_start(brow[:], b1[:].bitcast(F32R))
                ones512 = p_c.tile([1, 512], F32R, tag="ones")
                nc.sync.dma_start(ones512[:], ones512d[:].bitcast(F32R))

            # ls = exp(min(logit_scale, LOG_MAX)), broadcast to all partitions
            nc.vector.tensor_scalar_min(lsrow[:], lsrow[:], LOG_MAX)
            nc.scalar.activation(lsrow[:], lsrow[:], EXP)
            lsbc = p_c.tile([128, H], F32, tag="lsb")
            nc.gpsimd.partition_broadcast(lsbc[:], lsrow[:])
            hsbc = p_c.tile([128, H], F32, tag="hsb")
            nc.gpsimd.partition_broadcast(hsbc[:], hsrow[:])

            # ------------- qkv^T Q/K parts + per-pair norms ---------------
            # QT/KT[p, blk*L + m] = qkv^T row (blk*128+p) [+768 for K], col m
            # norms rows {0,32} col-block p = 1/||q|| heads (2p, 2p+1)
            # norms rows {64,96} col-block p = 1/||k|| heads (2p, 2p+1)
            QTd = p_dram.tile([C, L], F32, tag="qtd")
            KT = p_qk.tile([128, 6 * L], F32R, tag="kt")
            norms = p_n.tile([128, 6 * L], F32, tag="n")
            if debug:
                nc.gpsimd.memset(norms[:], 1.0)
            kscratch = p_dram.tile([H, L], F32, tag="ks")
            rkinv = p_n.tile([128, H * 8], F32, tag="rk")

            def qkv_jt(jt):
                """One j-tile (128 rows of qkv^T): matmul chain + eviction +
                squared-norms reduction into `norms`. Q rows bounce to DRAM
                (read back per head); K rows stay resident as matmul lhsT."""
                blk = jt % 6
                nrow = 0 if jt < 6 else 64
                if jt < 6:
                    dst = p_e.tile([128, L], F32R, tag="e", name=f"qtmp{jt}")
                else:
                    dst = KT[:, blk * L:(blk + 1) * L]
                sq = p_e.tile([128, L], F32R, tag="e", name=f"sq{jt}")
                pcol = 0 if jt < 6 else 128
                for lc in range(2):
                    ps = ps_q.tile([128, 512], F32, tag="q", name=f"qkps{jt}_{lc}")
                    for kt in range(NKT):
                        nc.tensor.matmul(
                            ps[:],
                            _r(wqk[(blk, kt)][:, pcol:pcol + 128]),
                            _r(xt6[:, kt * L + lc * 512: kt * L + lc * 512 + 512]),
                            start=(kt == 0),
                            stop=(kt == NKT - 1) and not has_b1,
                        )
                    if has_b1:
                        nc.tensor.matmul(
                            ps[:], _r(brow[:, jt * 128:(jt + 1) * 128]),
                            _r(ones512[:]), start=False, stop=True)
                    dsl = dst[:, lc * 512:lc * 512 + 512]
                    nc.vector.tensor_copy(dsl, ps[:])
                    nc.vector.tensor_tensor(sq[:, lc * 512:lc * 512 + 512],
                                            dsl, dsl, MULT)
                if jt < 6:
                    nc.sync.dma_start(QTd[blk * 128:(blk + 1) * 128, :],
                                      dst[:].bitcast(F32))
                for lc in range(2):
                    sps = ps_q.tile([33, 512], F32, tag="q", name=f"ssq{jt}_{lc}")
                    nc.tensor.matmul(sps[:], _r(bones[:]),
                                     _r(sq[:, lc * 512:lc * 512 + 512]),
                                     start=True, stop=True)
                    # fused eviction: ||.|| = sqrt(ssq) straight out of PSUM
                    nc.scalar.sqrt(
                        norms[nrow:nrow + 33, blk * L + lc * 512:
                              blk * L + lc * 512 + 512], sps[:])

            def pair(p):
                qkv_jt(p)          # Q pair p
                qkv_jt(6 + p)      # K pair p
                # finish Q rows; K rows bounce raw then recip at base 0
                reg = norms[0:33, p * L:(p + 1) * L]
                nc.vector.tensor_scalar_max(reg, reg, 1e-12)
                nc.vector.reciprocal_approx_fast(reg, reg)
                for i, krow in ((0, 64), (1, 96)):
                    h = 2 * p + i
                    nc.sync.dma_start(kscratch[h:h + 1, :],
                                      norms[krow:krow + 1, p * L:(p + 1) * L])
                    nc.sync.dma_start(
                        rkinv[:, h * 8:(h + 1) * 8]
                        .rearrange("p (a c) -> p a c", a=1),
                        kscratch[h:h + 1, :].rearrange("a (c p) -> p a c", p=128))
                kreg = rkinv[:, 2 * p * 8: 2 * p * 8 + 16]
                nc.vector.tensor_scalar_max(kreg, kreg, 1e-12)
                nc.vector.reciprocal_approx_fast(kreg, kreg)

            def half_pair(p, which):
                qkv_jt(p if which == 0 else 6 + p)
                if which == 0:
                    # finish Q rows (base 0): clamp + fast reciprocal; must
                    # be final before preamble(2p), which is emitted right
                    # after this half. Custom-DVE ops misbehave at partition
                    # bases 64/96 on HW, so K rows are NOT reciprocated in
                    # place; they bounce through DRAM as raw ||k||.
                    reg = norms[0:33, p * L:(p + 1) * L]
                    nc.vector.tensor_scalar_max(reg, reg, 1e-12)
                    nc.vector.reciprocal_approx_fast(reg, reg)
                else:
                    for i, krow in ((0, 64), (1, 96)):
                        h = 2 * p + i
                        nc.sync.dma_start(
                            kscratch[h:h + 1, :],
                            norms[krow:krow + 1, p * L:(p + 1) * L])
                        nc.sync.dma_start(
                            rkinv[:, h * 8:(h + 1) * 8]
                            .rearrange("p (a c) -> p a c", a=1),
                            kscratch[h:h + 1, :]
                            .rearrange("a (c p) -> p a c", p=128))
                    kreg = rkinv[:, 2 * p * 8: 2 * p * 8 + 16]
                    nc.vector.tensor_scalar_max(kreg, kreg, 1e-12)
                    nc.vector.reciprocal_approx_fast(kreg, kreg)

            pair(0)

            # ---------------- V rows, with ones column per head -----------
            # Vt[p, lt*780 + h*65 + d] = v[lt*128+p, h*64+d]; col h*65+64 = 1
            Vt = p_v.tile([128, NLT * H * 65], F32R, tag="v")
            for lt in range(NLT):
                base = lt * H * 65
                nc.sync.dma_start(
                    Vt[:, base:base + H * 65]
                    .rearrange("p (h e) -> p h e", e=65)[:, :, 64:65],
                    onescd[:].bitcast(F32R).rearrange("p (h o) -> p h o", o=1))
                for vo, nh in ((0, 8), (512, 4)):
                    nw = nh * 64
                    ps = ps_q.tile([128, 512], F32, tag="q", name=f"vps{lt}_{vo}")
                    for kt in range(NKT):
                        nc.tensor.matmul(
                            ps[:, 0:nw],
                            _r(xt6[:, kt * L + lt * 128: kt * L + lt * 128 + 128]),
                            _r(wv[kt][:, vo:vo + nw]),
                            start=(kt == 0),
                            stop=(kt == NKT - 1) and not has_b1,
                        )
                    if has_b1:
                        nc.tensor.matmul(
                            ps[:, 0:nw], _r(ones512[:, 0:128]),
                            _r(brow[:, 2 * C + vo: 2 * C + vo + nw]),
                            start=False, stop=True)
                    nc.vector.tensor_copy(
                        Vt[:, base + (vo // 64) * 65: base + (vo // 64) * 65 + nh * 65]
                        .rearrange("p (h e) -> p h e", e=65)[:, :, 0:64],
                        ps[:, 0:nw].rearrange("p (h d) -> p h d", d=64))

            # ---------------- attention, software-pipelined over heads ----
            # Engines run their instruction streams in order, so head h+1's
            # preamble (gpsimd broadcast + DVE scalar_tensor_tensor) must be
            # emitted BEFORE head h's postamble (which waits on h's full
            # attn@V chain) or the PE idles between heads.
            OTs = [p_ot.tile([128, L], F32R, tag=f"ot{i}", name=f"ot{i}")
                   for i in range(6)]
            qhats = {}

            def preamble(h):
                b = 64 * (h % 2)
                blk = h // 2
                # 1/||q|| row: col block h//2, row 0 (even h) / 32 (odd h).
                # HW partition_broadcast reads absolute partition 0, so odd
                # heads stage their row at partition 0 first.
                if h % 2 == 0:
                    rqsrc = norms[0:1, blk * L:(blk + 1) * L]
                else:
                    rqst = p_d.tile([1, L], F32, tag="d", name=f"rqst{h}")
                    nc.gpsimd.tensor_copy(rqst[:],
                                          norms[32:33, blk * L:(blk + 1) * L])
                    rqsrc = rqst[:]
                rqbc = p_b.tile([128, L], F32, tag="b", name=f"rqbc{h}")
                nc.gpsimd.partition_broadcast(rqbc[:], rqsrc)
                qtm = p_b.tile([128, L], F32R, tag="b", name=f"qtm{h}")
                nc.sync.dma_start(qtm[b:b + 64, :],
                                  QTd[blk * 128 + b: blk * 128 + b + 64,
                                      :].bitcast(F32R))
                qhat = p_b.tile([128, L], F32R, tag="b", name=f"qhat{h}")
                nc.vector.scalar_tensor_tensor(
                    qhat[b:b + 64, :], rqbc[b:b + 64, :], lsbc[b:b + 64, h:h + 1],
                    qtm[b:b + 64, :], MULT, MULT)
                qhats[h] = qhat

            def body(h):
                b = 64 * (h % 2)
                blk = h // 2
                qhat = qhats[h]
                ops = [ps_o.tile([65, 512], F32, tag="o", name=f"op{h}_{i}")
                       for i in range(2)]
                for mt in range(NLT):
                    et = p_e.tile([128, L], F32R, tag="e", name=f"et{h}_{mt}")
                    sps = ps_s.tile([128, L], F32, tag="s", name=f"sps{h}_{mt}")
                    for lc in range(2):
                        nc.tensor.matmul(
                            sps[:, lc * 512:lc * 512 + 512],
                            _r(KT[b:b + 64,
                                  blk * L + mt * 128: blk * L + mt * 128 + 128]),
                            _r(qhat[b:b + 64, lc * 512:lc * 512 + 512]),
                            start=True, stop=True)
                    nc.scalar.activation(et[:], sps[:], EXP,
                                         scale=rkinv[:, h * 8 + mt:h * 8 + mt + 1])
                    if debug and h == 0 and mt == 0:
                        nc.sync.dma_start(dbg["yet"][:], et[:].bitcast(F32))
                    for lc in range(2):
                        nc.tensor.matmul(
                            ops[lc][:],
                            _r(Vt[:, mt * H * 65 + h * 65: mt * H * 65 + (h + 1) * 65]),
                            _r(et[:, lc * 512:lc * 512 + 512]),
                            start=(mt == 0), stop=(mt == NLT - 1))
                return ops

            def postamble(h, ops):
                b = 64 * (h % 2)
                blk = h // 2
                dn = p_d.tile([1, L], F32, tag="d", name=f"dn{h}")
                for lc in range(2):
                    # native reciprocal: custom-DVE ops misread partition
                    # base 64 (the denominator row) on HW
                    nc.vector.reciprocal(
                        dn[0:1, lc * 512:lc * 512 + 512], ops[lc][64:65, :])
                if debug and h == 0:
                    nc.sync.dma_start(dbg["ydn"][:], dn[:])
                obc = p_b.tile([128, L], F32, tag="b", name=f"obc{h}")
                nc.gpsimd.partition_broadcast(obc[:], dn[:])
                for lc in range(2):
                    nc.vector.scalar_tensor_tensor(
                        OTs[blk][b:b + 64, lc * 512:lc * 512 + 512],
                        obc[b:b + 64, lc * 512:lc * 512 + 512],
                        hsbc[b:b + 64, h:h + 1],
                        ops[lc][0:64, :], MULT, MULT)

            # interleave: qkv pair p+1 is emitted between the bodies of
            # pair p's heads so PE alternates qkv chains with attention and
            # ACT's exp stream starts as early as possible
            preamble(0)
            if debug:
                nc.sync.dma_start(dbg["yqhat"][0:64, :],
                                  qhats[0][0:64, :].bitcast(F32))
            preamble(1)
            load_wqk_pair(1)
            postq = []
            for p in range(1, 6):
                if p + 1 < 6:
                    load_wqk_pair(p + 1)
                for i in range(2):
                    half_pair(p, i)
                    h = 2 * (p - 1) + i
                    ops = body(h)
                    if h + 2 < H:
                        preamble(h + 2)
                    if postq:
                        postamble(*postq.pop(0))
                    postq.append((h, ops))
            for h in (10, 11):
                ops = body(h)
                if postq:
                    postamble(*postq.pop(0))
                postq.append((h, ops))
            while postq:
                postamble(*postq.pop(0))

            if debug:
                for i in range(6):
                    nc.sync.dma_start(dbg["yot"][:, i * L:(i + 1) * L],
                                      OTs[i][:].bitcast(F32))
                nc.sync.dma_start(dbg["ykt"][:], KT[:].bitcast(F32))
                nc.sync.dma_start(dbg["yvt"][:], Vt[:].bitcast(F32))
                nc.sync.dma_start(dbg["ynorms"][:], norms[:])
                nc.sync.dma_start(dbg["yrk"][:], rkinv[:])

            # ---------------- output projection -----------------
            owts = []
            for ct in range(NKT):
                t = p_w.tile([128, C], F32R, tag="w", name=f"owt{ct}")
                nc.sync.dma_start(t[:],
                                  owt[ct * 128:(ct + 1) * 128, :].bitcast(F32R))
                owts.append(t)
            for lt in range(NLT):
                fout = p_b.tile([128, C], F32, tag="b", name=f"fout{lt}")
                for n0, nw in ((0, 512), (512, 256)):
                    ps = ps_q.tile([128, 512], F32, tag="q", name=f"fps{lt}_{n0}")
                    for ct in range(NKT):
                        nc.tensor.matmul(
                            ps[:, 0:nw],
                            _r(OTs[ct][:, lt * 128: lt * 128 + 128]),
                            _r(owts[ct][:, n0:n0 + nw]),
                            start=(ct == 0), stop=(ct == NKT - 1))
                    nc.vector.tensor_copy(fout[:, n0:n0 + nw], ps[:, 0:nw])
                nc.sync.dma_start(y[lt * 128:(lt + 1) * 128, :], fout[:])


_PROG_CACHE = {}


def _get_program(has_b1, debug=False):
    key = (has_b1, debug)
    if key not in _PROG_CACHE:
        nc = bacc.Bacc("TRN2", target_bir_lowering=False, debug=False,
                       enable_asserts=False)
        build(nc, has_b1, debug=debug)
        nc.compile()
        _PROG_CACHE[key] = nc
    return _PROG_CACHE[key]


def kernel(x, in_proj_weight, in_proj_bias, logit_scale, head_scale, out_w,
           out_b):
    x = np.asarray(x, np.float32)
    in_proj_weight = np.asarray(in_proj_weight, np.float32)
    in_proj_bias = np.asarray(in_proj_bias, np.float32)
    logit_scale = np.asarray(logit_scale, np.float32)
    head_scale = np.asarray(head_scale, np.float32)
    out_w = np.asarray(out_w, np.float32)
    out_b = np.asarray(out_b, np.float32)

    n_cores = x.shape[1]
    assert x.shape == (L, n_cores, C)

    has_b1 = bool(np.any(in_proj_bias))
    nc = _get_program(has_b1)

    xt_all = np.ascontiguousarray(np.transpose(x, (1, 2, 0)))      # [N, C, L]
    wt = np.ascontiguousarray(in_proj_weight.T)                    # [C, 3C]
    owt = np.ascontiguousarray(out_w.T)                            # [C, C]
    ls2 = np.ascontiguousarray(logit_scale.reshape(1, H))
    hs2 = np.ascontiguousarray(head_scale.reshape(1, H))

    bones_np = np.zeros((128, 33), np.float32)
    bones_np[0:64, 0] = 1.0
    bones_np[64:128, 32] = 1.0
    onesc_np = np.ones((128, H), np.float32)

    in_maps = []
    for i in range(n_cores):
        m = {"xt": xt_all[i], "wt": wt, "owt": owt, "lsc": ls2, "hsc": hs2,
             "bones": bones_np, "onesc": onesc_np}
        if has_b1:
            m["b1"] = np.ascontiguousarray(in_proj_bias.reshape(1, 3 * C))
            m["ones512"] = np.ones((1, 512), np.float32)
        in_maps.append(m)

    res = bass_utils.run_bass_kernel_spmd(nc, in_maps,
                                          core_ids=list(range(n_cores)))
    yout = np.stack([r["y"] for r in res.results], axis=1)         # [L, N, C]
    if np.any(out_b):
        yout = yout + out_b
    return np.ascontiguousarray(yout.astype(np.float32))

